# revision 56
# baseline (speedup 1.0000x reference)
"""Causal self-attention on 8 TRN2 NeuronCores.

Sharding: core c = (batch b = c // 2, head-group g = c % 2).
Each core handles one batch and 8 of the 16 heads:
  - QKV projection for its 512 q/k/v feature slices (transposed layout)
  - causal attention for its 8 heads
  - partial output projection (its 512 rows of W_out)
Host sums the two partials per batch and adds b_out.

All TensorE matmuls run in bf16; softmax runs in f32 (exp on ScalarE,
normalization via ones-column sums + VectorE reciprocal).

Scores matmuls have K=64 contraction, so even/odd heads of a pair are laid
out at SBUF partitions 0-63 / 64-127 and issued back-to-back: the PE runs
them concurrently in the top/bottom halves of the systolic array (row
tiling), writing different PSUM banks of one shared [128, 2048] tile that a
single ScalarE exp then evacuates.

Scheduling: ScalarE exp is the attention-phase limiter (~1.15us per full
score group vs ~0.64us of PE work), so the kernel starts exp as early as
possible, spreads the expensive qc=3 pairs among the qc=1 pairs
((0,*)x4, (1,k)/(3,k) interleaved, (2,*)x4 last), and keeps the PE stream
stocked with non-attention fill work (V chunks, deferred qk units, output
projection, split softmax-division units) popping one per score group.
W_qkv's q|k columns load first (masks are tiny and go before), x streams
in quarter-T slices, and dummy-matmul warmup/padding holds the PE HAM
clock gate at full rate through the DMA-paced start. AV matmuls trail
their score group by ~6 groups so they never wait on exp. Diagonal score
tiles stream only live columns through the score and AV matmuls, exp only
live columns, and mask-multiply only the [128,128] triangle block.
"""

import numpy as np
import ml_dtypes

B, T, D, H = 4, 2048, 1024, 16
HG = 2            # head groups (tensor-parallel factor)
HL = H // HG      # 8 heads per core
HD = D // H       # 64
DG = HL * HD      # 512 features per group
SCALE = 1.0 / float(np.sqrt(HD))
NCORES = 8
TCH = T // 128    # 16 time chunks of 128
NQC = T // 512    # 4 query chunks of 512
VW = HD + 1       # 65: v columns + ones column per head

bf16 = ml_dtypes.bfloat16

_CACHE = {}


def _split_multi_waits(nc, mybir):
    """The TPB instruction encoding has a single wait slot; this walrus build
    rejects instructions carrying more than one sync wait. Hoist extra waits
    onto standalone EventSemaphore instructions on the same engine. Tile's
    schedule is a valid serialization (waits only reference earlier-ordered
    work on other streams), so blocking the issuing stream at the same point
    cannot deadlock."""
    SKIP = ("InstTriggerDma", "InstCollectiveCompute")
    ENG_PREFIX = {
        "EngineType.PE": "PE_",
        "EngineType.Activation": "Activation_",
        "EngineType.DVE": "DVE_",
        "EngineType.Pool": "Pool_",
        "EngineType.SP": "SP_",
    }
    for f in nc.m.functions:
        for blk in f.blocks:
            out = []
            changed = False
            for inst in blk.instructions:
                si = getattr(inst, "sync_info", None)
                ow = list(si.on_wait) if si is not None and si.on_wait else []
                if len(ow) > 1 and type(inst).__name__ not in SKIP:
                    # a wait on the instruction's own engine counter is
                    # trivially satisfied (in-order queues; the increment
                    # comes from an earlier own-engine instruction, else
                    # Tile's schedule would already deadlock) — drop it
                    # rather than burn a queue slot on it
                    pref = ENG_PREFIX.get(str(inst.engine))
                    if pref:
                        keep = [w for w in ow
                                if not (getattr(w, "ant_name", "") or "").startswith(pref)]
                        if keep:
                            if len(keep) != len(ow):
                                changed = True
                            ow = keep
                            inst.sync_info = mybir.SyncInfo(
                                on_wait=list(ow),
                                on_update=list(si.on_update) if si.on_update else [],
                            )
                            si = inst.sync_info
                if len(ow) > 1 and type(inst).__name__ not in SKIP:
                    for i, w in enumerate(ow[:-1]):
                        out.append(mybir.InstEventSemaphore(
                            name=f"{inst.name}_hw{i}",
                            engine=inst.engine,
                            sync_info=mybir.SyncInfo(on_wait=[w], on_update=[]),
                            bass_nofuse=True,
                        ))
                    inst.sync_info = mybir.SyncInfo(
                        on_wait=[ow[-1]],
                        on_update=list(si.on_update) if si.on_update else [],
                    )
                    changed = True
                out.append(inst)
            if changed:
                blk.instructions = out


def _build_bass():
    import concourse.bass as bass
    import concourse.mybir as mybir
    import concourse.tile as tile
    from contextlib import ExitStack

    dt = mybir.dt
    f32 = dt.float32
    bf = dt.bfloat16

    nc = bass.Bass()
    xT_d = nc.declare_dram_parameter("xT", [D, T], bf, isOutput=False)
    wqk_d = nc.declare_dram_parameter("wqk", [D, 2 * DG], bf, isOutput=False)
    wv_d = nc.declare_dram_parameter("wv", [D, DG], bf, isOutput=False)
    wo_d = nc.declare_dram_parameter("wo", [DG, D], bf, isOutput=False)
    masks_d = nc.declare_dram_parameter("masks", [128, 256], bf, isOutput=False)
    oh_d = nc.declare_dram_parameter("oh", [8, 8 * 64], bf, isOutput=False)
    out_d = nc.declare_dram_parameter("out", [T, D], bf, isOutput=True)

    with tile.TileContext(nc) as tc, ExitStack() as ctx:
        const = ctx.enter_context(tc.tile_pool(name="const", bufs=1))
        psum = ctx.enter_context(tc.tile_pool(name="psum", bufs=2, space="PSUM"))
        ptp = ctx.enter_context(tc.tile_pool(name="ptp", bufs=7))
        stp = ctx.enter_context(tc.tile_pool(name="stp", bufs=10))
        small = ctx.enter_context(tc.tile_pool(name="small", bufs=3))

        # ---- resident tensors --------------------------------------------
        xT_sb = const.tile([128, 8, T], bf)          # x[b].T   (feature-major)
        wqk_sb = const.tile([128, 8, 2 * DG], bf)    # W_qkv q|k columns
        wv_sb = const.tile([128, 8, DG], bf)         # W_qkv v columns
        wo_sb = const.tile([128, 4, D], bf)          # W_out rows for group
        qkT_sb = const.tile([128, 8, T], bf)         # [q^T | k^T]  (feature-major)
        vn_sb = const.tile([128, TCH, HL * VW], bf)  # V natural + ones column
        at_sb = const.tile([128, 4, T], bf)          # A^T (normalized attn out)
        masks_sb = const.tile([128, 256], bf)        # triangle mask x2 heads
        oh_sb = const.tile([8, 8 * 64], bf)          # one-hot lhsT for PE row-broadcast
        scr_sb = const.tile([128, 512], bf)          # warmup scratch (memset)

        # Load order is tuned so attention starts early and nothing stalls:
        # masks (tiny) first, then the full q|k weight block (it gates the
        # first score group), then x in quarter-T slices interleaved with wv
        # so V-projection streams right behind the qk units. wo/oh are
        # needed only ~100us in.
        nc.sync.dma_start(out=masks_sb, in_=masks_d[:, :])
        # q-halves of wqk + the first x quarter land first: the first score
        # unit's contraction steps stream with their arrival
        for c in range(0, 8, 2):
            nc.sync.dma_start(out=wqk_sb[:, c, 0:DG], in_=wqk_d[c * 128:(c + 1) * 128, 0:DG])
            nc.gpsimd.dma_start(out=wqk_sb[:, c + 1, 0:DG], in_=wqk_d[(c + 1) * 128:(c + 2) * 128, 0:DG])
            nc.sync.dma_start(out=xT_sb[:, c, 0:512], in_=xT_d[c * 128:(c + 1) * 128, 0:512])
            nc.gpsimd.dma_start(out=xT_sb[:, c + 1, 0:512], in_=xT_d[(c + 1) * 128:(c + 2) * 128, 0:512])
        for c in range(0, 8, 2):
            nc.sync.dma_start(out=wqk_sb[:, c, DG:2 * DG], in_=wqk_d[c * 128:(c + 1) * 128, DG:2 * DG])
            nc.gpsimd.dma_start(out=wqk_sb[:, c + 1, DG:2 * DG], in_=wqk_d[(c + 1) * 128:(c + 2) * 128, DG:2 * DG])
        for c in range(0, 8, 2):
            nc.sync.dma_start(out=wv_sb[:, c, :], in_=wv_d[c * 128:(c + 1) * 128, :])
            nc.gpsimd.dma_start(out=wv_sb[:, c + 1, :], in_=wv_d[(c + 1) * 128:(c + 2) * 128, :])
        for q in range(1, 4):
            for c in range(0, 8, 2):
                nc.sync.dma_start(out=xT_sb[:, c, q * 512:(q + 1) * 512],
                                  in_=xT_d[c * 128:(c + 1) * 128, q * 512:(q + 1) * 512])
                nc.gpsimd.dma_start(out=xT_sb[:, c + 1, q * 512:(q + 1) * 512],
                                    in_=xT_d[(c + 1) * 128:(c + 2) * 128, q * 512:(q + 1) * 512])
        nc.sync.dma_start(out=oh_sb, in_=oh_d[:, :])
        for c in range(4):
            nc.gpsimd.dma_start(out=wo_sb[:, c, :], in_=wo_d[c * 128:(c + 1) * 128, :])

        # ---- PE warmup ---------------------------------------------------
        # ~12 dummy matmuls (~5us cold) on scratch keep the PE busy through
        # the HAM SHORT window while input DMA streams, so real work runs at
        # 2.4 GHz instead of 1.2 from the first matmul on.
        nc.vector.memset(scr_sb, 0.0)
        warm_n = [0]

        def warm_mm():
            pw = psum.tile([128, 1024], f32, tag="s", name=f"warm{warm_n[0]}")
            warm_n[0] += 1
            nc.tensor.matmul(pw[:, 0:512], lhsT=scr_sb[:, 0:128], rhs=scr_sb,
                             start=True, stop=True)

        def warm_mm2():
            # mm512-tag warm: its pool dependency is a fill unit's cast (fast),
            # not the score-psum rotation's exp wait — usable as boundary cover
            pw = psum.tile([128, 512], f32, tag="mm512",
                           name=f"warm2_{warm_n[0]}")
            warm_n[0] += 1
            nc.tensor.matmul(pw, lhsT=scr_sb[:, 0:128], rhs=scr_sb,
                             start=True, stop=True)

        for w in range(16):
            warm_mm()

        def qkv_v_chunk(tn):
            pv = psum.tile([128, 512], f32, tag="mm512", name=f"pv{tn}")
            for k in range(8):
                nc.tensor.matmul(
                    pv,
                    lhsT=xT_sb[:, k, tn * 128:(tn + 1) * 128],
                    rhs=wv_sb[:, k, :],
                    start=(k == 0), stop=(k == 7),
                )
            vrow = vn_sb[:, tn, :].rearrange("p (h e) -> p h e", e=VW)
            nc.vector.tensor_copy(
                out=vrow[:, :, 0:HD],
                in_=pv.rearrange("p (h e) -> p h e", e=HD),
            )
            nc.vector.memset(vrow[:, :, HD:VW], 1.0)

        qk_psums = {}

        def qk_half(m, n, half, pad=0):
            """Half of one [128 feat x 512 time] q|k projection block."""
            if half == 0:
                qk_psums[(m, n)] = psum.tile([128, 512], f32, tag="mm512",
                                             name=f"pq{m}_{n}")
            pq = qk_psums[(m, n)]
            for k in range(4 * half, 4 * half + 4):
                nc.tensor.matmul(
                    pq,
                    lhsT=wqk_sb[:, k, m * 128:(m + 1) * 128],
                    rhs=xT_sb[:, k, n * 512:(n + 1) * 512],
                    start=(k == 0), stop=(k == 7),
                )
                # pad>0 on the DMA-paced first units: dummies run during the
                # arrival stalls so the HAM clock gate sees a busy PE
                for _ in range(pad):
                    warm_mm()
            if half == 1:
                nc.vector.tensor_copy(
                    out=qkT_sb[:, m, n * 512:(n + 1) * 512], in_=pq)

        def qk_unit(m, n, pad=0):
            qk_half(m, n, 0, pad)
            qk_half(m, n, 1, pad)

        # ---- attention ---------------------------------------------------
        def attn_pair(qc, p, fill=None):
            """Scores + AV for head pair p of query chunk qc. Each score group
            is one kc for both heads of the pair ([128, 1024] psum, two
            concurrent row-tiled K=64 matmuls). AV matmuls trail their score
            group by ~6 groups (ptp bufs=7) so they never wait on ScalarE's
            exp; one fill unit pops per group to absorb the exp-paced gap."""
            nkc = 4 * qc + 4
            h0, h1 = 2 * p, 2 * p + 1
            qsl0 = qkT_sb[0:64, p, qc * 512:(qc + 1) * 512]
            qsl1 = qkT_sb[64:128, p, qc * 512:(qc + 1) * 512]
            pts = []
            pav0 = psum.tile([VW, 512], f32, tag="av", name=f"pav0_{qc}_{p}")
            pav1 = psum.tile([VW, 512], f32, tag="av", name=f"pav1_{qc}_{p}")

            def av_group(kc):
                # diagonal tiles only have live queries >= 128*dg: stream
                # just those columns (the dead ones contribute exactly 0)
                lo = max(0, 128 * (kc - (nkc - 4)))
                nc.tensor.matmul(
                    pav0[:, lo:512],
                    lhsT=vn_sb[:, kc, h0 * VW:(h0 + 1) * VW],
                    rhs=pts[kc][:, lo:512],
                    start=(kc == 0), stop=(kc == nkc - 1),
                )
                nc.tensor.matmul(
                    pav1[:, lo:512],
                    lhsT=vn_sb[:, kc, h1 * VW:(h1 + 1) * VW],
                    rhs=pts[kc][:, 512 + lo:1024],
                    start=(kc == 0), stop=(kc == nkc - 1),
                )

            for kc in range(nkc):
                dg = kc - (nkc - 4)  # 0..3 on the masked diagonal band
                lo = max(0, 128 * dg)
                ps = psum.tile([128, 1024], f32, tag="s", name=f"ps{qc}_{p}_{kc}")
                nc.tensor.matmul(
                    ps[:, lo:512],
                    lhsT=qkT_sb[0:64, 4 + p, kc * 128:(kc + 1) * 128],
                    rhs=qsl0[:, lo:512], start=True, stop=True,
                )
                nc.tensor.matmul(
                    ps[:, 512 + lo:1024],
                    lhsT=qkT_sb[64:128, 4 + p, kc * 128:(kc + 1) * 128],
                    rhs=qsl1[:, lo:512], start=True, stop=True,
                )
                if kc % 2 == 0 and kc >= 6:
                    # batch AV for two kc at once: fewer PE array-mode
                    # switches between K=64 score tiles and K=128 AV tiles
                    av_group(kc - 6)
                    av_group(kc - 5)
                if kc > 0 and fill:
                    fill.pop(0)()  # PE fill work for exp-paced stalls
                pt = ptp.tile([128, 1024], bf, tag="pt", name=f"pt{qc}_{p}_{kc}")
                if dg < 0:
                    nc.scalar.activation(
                        out=pt, in_=ps,
                        func=mybir.ActivationFunctionType.Exp, scale=SCALE,
                    )
                else:
                    # diagonal tile: cols [0:128dg) dead (and never read —
                    # the AV matmul streams only live columns),
                    # [128dg:128dg+128) triangular, rest live. Exp live cols
                    # only; mask-mul only the triangle block.
                    ptv = pt.rearrange("p (h c) -> p h c", c=512)
                    psv = ps.rearrange("p (h c) -> p h c", c=512)
                    if dg > 0:
                        nc.scalar.activation(
                            out=ptv[:, :, lo:512], in_=psv[:, :, lo:512],
                            func=mybir.ActivationFunctionType.Exp, scale=SCALE,
                        )
                    else:
                        nc.scalar.activation(
                            out=pt, in_=ps,
                            func=mybir.ActivationFunctionType.Exp, scale=SCALE,
                        )
                    nc.vector.tensor_mul(
                        out=ptv[:, :, lo:lo + 128], in0=ptv[:, :, lo:lo + 128],
                        in1=masks_sb.rearrange("p (h c) -> p h c", c=128),
                    )
                pts.append(pt)
            for kc in range(max(nkc - 6, 0), nkc):
                av_group(kc)

            out = []
            for h, pav in ((h0, pav0), (h1, pav1)):
                stage = stp.tile([VW, 512], bf, tag="stage", bufs=20,
                                 name=f"st{qc}_{h}")
                nc.vector.tensor_copy(out=stage, in_=pav)
                out.append((stage, pav))
            return out

        colls = {}
        stages = {}

        def attn(qc, p, fill=None):
            colls[(qc, p)] = stp.tile([8, 128], f32, tag="coll", bufs=10,
                                      name=f"coll{qc}_{p}")
            st0, st1 = attn_pair(qc, p, fill)
            stages[(qc, 2 * p)], stages[(qc, 2 * p + 1)] = st0[0], st1[0]
            for h, (st, pav) in ((2 * p, st0), (2 * p + 1, st1)):
                # sums row [1, 512] -> 4 partitions x 128 so the reciprocal
                # runs wide (per-lane free count 128, not 512)
                nc.gpsimd.dma_start(
                    out=colls[(qc, p)][4 * (h % 2):4 * (h % 2) + 4, :],
                    in_=st[HD:VW, :].rearrange("o (a b) -> o a b", b=128),
                )

        def divisions_pair(qc, p):
            """Normalize the two heads of pair (qc, p) into at_sb. Split in
            two fill units — the DVE reciprocal chain (a) pops a few groups
            before the PE replicate+apply (b) so its latency hides behind
            real work instead of head-blocking the PE queue."""
            state = {}

            def emit_recip():
                coll = colls[(qc, p)]
                rc = stp.tile([8, 128], f32, tag="rcoll", bufs=4,
                              name=f"rc{qc}_{p}")
                nc.vector.reciprocal(rc, coll)
                rcb = stp.tile([8, 128], bf, tag="rcollb", bufs=4,
                               name=f"rcb{qc}_{p}")
                nc.vector.tensor_copy(out=rcb, in_=rc)
                state["rcb"] = rcb

            def emit_apply():
                rcb = state["rcb"]
                for h in (2 * p, 2 * p + 1):
                    # replicate head h's reciprocal rows across 64 partitions
                    # via a one-hot stationary matmul
                    prb = psum.tile([64, 512], f32, tag="mm512",
                                    name=f"prb{qc}_{h}")
                    for a in range(4):
                        j = 4 * (h % 2) + a
                        nc.tensor.matmul(
                            prb[:, a * 128:(a + 1) * 128],
                            lhsT=oh_sb[:, j * 64:(j + 1) * 64],
                            rhs=rcb[:, :],
                            start=True, stop=True,
                        )
                    if h % 2 == 0:
                        nc.vector.tensor_mul(
                            out=at_sb[0:64, h // 2, qc * 512:(qc + 1) * 512],
                            in0=stages[(qc, h)][0:HD, :], in1=prb,
                        )
                    else:
                        dtmp = small.tile([64, 512], bf, tag="dtmp")
                        nc.vector.tensor_mul(
                            out=dtmp, in0=stages[(qc, h)][0:HD, :], in1=prb)
                        nc.sync.dma_start(
                            out=at_sb[64:128, h // 2, qc * 512:(qc + 1) * 512],
                            in_=dtmp,
                        )
            return emit_recip, emit_apply

        def outproj_unit(qj, dn, sq=False):
            def emit():
                po = psum.tile([128, 512], f32, tag="mm512",
                               name=f"po{qj}_{dn}")
                for kc in range(4):
                    nc.tensor.matmul(
                        po,
                        lhsT=at_sb[:, kc, qj * 128:(qj + 1) * 128],
                        rhs=wo_sb[:, kc, dn * 512:(dn + 1) * 512],
                        start=(kc == 0), stop=(kc == 3),
                    )
                ost = small.tile([128, 512], bf, tag="ost")
                nc.vector.tensor_copy(out=ost, in_=po)
                # ScalarE is idle by the final projection — draining the
                # last writes on its queue halves the end-of-kernel wait
                eng = nc.scalar if sq else nc.sync
                eng.dma_start(
                    out=out_d[qj * 128:(qj + 1) * 128,
                              dn * 512:(dn + 1) * 512],
                    in_=ost,
                )
            return emit

        def outproj_units(qc, sq_last=0):
            return [outproj_unit(qj, dn,
                                 sq=(qj - 4 * qc) * 2 + dn >= 8 - sq_last)
                    for qj in range(4 * qc, 4 * qc + 4) for dn in range(2)]

        def vf(tn):
            return lambda: qkv_v_chunk(tn)

        def qkh(m, n, half):
            return lambda: qk_half(m, n, half)

        # ---- schedule ----------------------------------------------------
        # qc attention rides right behind its own q/k units (n-major); V
        # chunks, deferred qk n=3 halves, output projection, and
        # pair-granular divisions are spread through the pairs as fills so
        # the PE always has work while ScalarE exps.
        # Pair order interleaves the expensive qc=3 pairs among qc=1 pairs —
        # (0,*)x4, then (1,k),(3,k) for k=0..3, then (2,*)x4 — so ScalarE's
        # exp load (which grows with qc) is spread across the whole kernel
        # instead of crunching at the end. qk units and divisions pop as
        # fills just ahead of the pair that consumes them.
        op0 = outproj_units(0)
        op1 = outproj_units(1)
        op3 = outproj_units(3)
        dpr = {}
        dpb = {}
        for qc in range(4):
            for p in range(4):
                dpr[(qc, p)], dpb[(qc, p)] = divisions_pair(qc, p)
        wf = warm_mm
        f0 = [
            [vf(2), vf(3)],
            [vf(4), vf(5)],
            [vf(6), qkh(0, 1, 0), qkh(0, 1, 1)],
            [vf(7), qkh(4, 1, 0), qkh(4, 1, 1)],
        ]
        for p in range(4):
            qk_unit(p, 0, pad=2 if p == 0 else 0)
            qk_unit(4 + p, 0, pad=2 if p == 0 else 0)
            if p == 0:
                qkv_v_chunk(0)
                qkv_v_chunk(1)
            attn(0, p, f0[p])
        for tn in range(8, 12):
            qkv_v_chunk(tn)
        f1 = [
            [qkh(4, 2, 0), qkh(4, 2, 1), qkh(0, 3, 0), qkh(0, 3, 1),
             vf(12), vf(13), vf(14)],
            [dpr[(1, 0)], qkh(5, 2, 0), qkh(5, 2, 1), dpb[(1, 0)],
             qkh(1, 3, 0), qkh(1, 3, 1), dpr[(3, 0)]],
            [dpr[(1, 1)], qkh(6, 2, 0), qkh(6, 2, 1), dpb[(1, 1)],
             qkh(2, 3, 0), qkh(2, 3, 1), dpr[(3, 1)]],
            [dpr[(1, 2)], qkh(7, 2, 0), qkh(7, 2, 1), dpb[(1, 2)],
             qkh(3, 3, 0), qkh(3, 3, 1), dpr[(3, 2)]],
        ]
        f3 = [
            [qkh(4, 3, 0), qkh(4, 3, 1), vf(15), dpr[(0, 0)], dpr[(0, 1)],
             dpb[(0, 0)], dpb[(0, 1)], dpr[(0, 2)], dpr[(0, 3)],
             qkh(0, 2, 0), qkh(0, 2, 1), qkh(1, 1, 0), qkh(1, 1, 1),
             qkh(5, 1, 0), qkh(5, 1, 1)],
            [dpb[(0, 2)], dpb[(0, 3)], dpb[(3, 0)], qkh(5, 3, 0),
             qkh(5, 3, 1), qkh(1, 2, 0), qkh(1, 2, 1),
             qkh(2, 1, 0), qkh(2, 1, 1), qkh(6, 1, 0), qkh(6, 1, 1),
             op0[0], op0[1], op0[2], op0[3]],
            [qkh(6, 3, 0), qkh(6, 3, 1), dpb[(3, 1)], qkh(2, 2, 0),
             qkh(2, 2, 1), qkh(3, 1, 0), qkh(3, 1, 1),
             qkh(7, 1, 0), qkh(7, 1, 1),
             op0[4], op0[5], op0[6], op0[7]],
            [qkh(7, 3, 0), qkh(7, 3, 1), dpb[(3, 2)], qkh(3, 2, 0),
             qkh(3, 2, 1)],
        ]
        for k in range(4):
            if k == 0:
                warm_mm2()
                warm_mm2()
            attn(1, k, f1[k])
            attn(3, k, f3[k])
        f2 = [
            [dpr[(1, 3)], dpr[(3, 3)], dpb[(1, 3)], wf, op1[0],
             dpb[(3, 3)], op1[1], op1[2], op1[3]],
            [op1[4], op1[5], op1[6], dpr[(2, 0)], op1[7],
             op3[0], dpb[(2, 0)], op3[1], op3[2]],
            [op3[3], op3[4], dpr[(2, 1)], op3[5], dpb[(2, 1)]],
            [op3[6], op3[7], dpr[(2, 2)], wf, dpb[(2, 2)]],
        ]
        for k in range(4):
            warm_mm2()
            warm_mm2()
            attn(2, k, f2[k])
        # keep the PE busy (and the clock gate warm) through the last
        # pair's softmax-division chain before the final output projection
        for _ in range(6):
            warm_mm()
        dpr[(2, 3)]()
        for _ in range(12):
            warm_mm()
        dpb[(2, 3)]()
        for u in outproj_units(2, sq_last=4):
            u()

    _split_multi_waits(nc, mybir)
    return nc


def _make_masks():
    kl = np.arange(128)[:, None]
    ql = np.arange(128)[None, :]
    tri = (ql >= kl).astype(np.float32)
    # the same within-tile triangle masks every diagonal block; duplicated
    # for the two heads packed side by side in each [128, 1024] score group
    return np.concatenate([tri, tri], axis=1).astype(bf16)  # [128, 256]


def _make_in_maps(x, W_qkv, W_out):
    masks = _make_masks()
    # oh[k, 64*j + m] = (k == j): one-hot stationary used to replicate
    # reciprocal rows across partitions on the TensorEngine
    oh = np.zeros((8, 8, 64), np.float32)
    for j in range(8):
        oh[j, j, :] = 1.0
    oh = oh.reshape(8, 8 * 64).astype(bf16)
    in_maps = []
    for c in range(NCORES):
        b, g = divmod(c, 2)
        xT = np.ascontiguousarray(x[b].T).astype(bf16)
        wq = W_qkv[:, g * DG:(g + 1) * DG]
        wk = W_qkv[:, D + g * DG:D + (g + 1) * DG]
        wv = W_qkv[:, 2 * D + g * DG:2 * D + (g + 1) * DG]
        wqk = np.concatenate([wq, wk], axis=1).astype(bf16)
        wo = W_out[g * DG:(g + 1) * DG, :].astype(bf16)
        in_maps.append({
            "xT": xT,
            "wqk": wqk,
            "wv": np.ascontiguousarray(wv).astype(bf16),
            "wo": np.ascontiguousarray(wo),
            "masks": masks,
            "oh": oh,
        })
    return in_maps


def _np_fallback(x, W_qkv, b_qkv, W_out, b_out):
    out = np.empty((B, T, D), np.float32)
    qkv = x.reshape(B * T, D) @ W_qkv + b_qkv
    q, k, v = np.split(qkv.reshape(B, T, 3 * D), 3, axis=-1)

    def heads(z):
        return z.reshape(B, T, H, HD).transpose(0, 2, 1, 3)

    q, k, v = heads(q), heads(k), heads(v)
    causal = np.tril(np.ones((T, T), dtype=bool))
    acc = np.empty((B, H, T, HD), np.float32)
    for bi in range(B):
        for h in range(H):
            s = (q[bi, h] @ k[bi, h].T) * np.float32(SCALE)
            s = np.where(causal, s, -np.inf)
            s -= s.max(axis=-1, keepdims=True)
            p = np.exp(s)
            p /= p.sum(axis=-1, keepdims=True)
            acc[bi, h] = p @ v[bi, h]
    a = acc.transpose(0, 2, 1, 3).reshape(B, T, D)
    for bi in range(B):
        out[bi] = a[bi] @ W_out + b_out
    return out


def run(x, W_qkv, b_qkv, W_out, b_out, trace=False, trace_kwargs=None):
    from concourse import bass_utils

    x = np.asarray(x, np.float32)
    W_qkv = np.asarray(W_qkv, np.float32)
    b_qkv = np.asarray(b_qkv, np.float32)
    W_out = np.asarray(W_out, np.float32)
    b_out = np.asarray(b_out, np.float32)

    # the on-device kernel assumes b_qkv == 0 (true for this problem
    # family; b_out is host-added). Fall back if not.
    if np.any(b_qkv):
        return _np_fallback(x, W_qkv, b_qkv, W_out, b_out), None

    if "nc" not in _CACHE:
        _CACHE["nc"] = _build_bass()
    nc = _CACHE["nc"]

    in_maps = _make_in_maps(x, W_qkv, W_out)
    kw = dict(trace=trace)
    if trace_kwargs:
        kw.update(trace_kwargs)
    res = bass_utils.run_bass_kernel_spmd(nc, in_maps, list(range(NCORES)), **kw)

    out = np.empty((B, T, D), np.float32)
    for b in range(B):
        out[b] = (np.asarray(res.results[2 * b]["out"], np.float32)
                  + np.asarray(res.results[2 * b + 1]["out"], np.float32)
                  + b_out)
    return out, res


def kernel(x, W_qkv, b_qkv, W_out, b_out):
    out, _ = run(x, W_qkv, b_qkv, W_out, b_out, trace=False)
    return out
